# revision 37
# baseline (speedup 1.0000x reference)
"""Trainium2 Bass kernel for nn_MultiHeadAttention (B=4, N=2048, E=768, H=8).

Sharding: 8 cores = 4 batches x 2 head-halves (4 heads each). Each core
computes QKV projections for its head slice, attention, and a partial output
projection; the host sums the two partials per batch and adds bo.

Per-core layout is fully "transposed" (feature-dim on partitions) so that no
on-chip transposes are needed anywhere:
  q/k are drained to fp8e4 as [96, head, token] and DMA-reshuffled into
  [48, 2, head, token]; the scores matmul runs in fp8 DoubleRow perf mode
  (two 48-deep k-tiles per pass at 0.5 cycles/row - 2x the bf16 rate, with
  q/k quantization noise averaging out across the d=96 contraction).
  softmax over the partition dim (j) with no max-subtraction (scores are
  small: |s| < ~3), denominator computed by a ones-column augmented V in the
  PV matmul, normalization by a DRAM-bounce-broadcast reciprocal. PV and all
  projections stay bf16: fp8 there injects per-key/per-channel noise that
  does NOT average out and lands at or above the 2e-2 error budget.

ISA constraint respected throughout: DVE TensorScalar/TensorCopy/TensorTensor
instructions fit only ONE sync wait, so every DVE op is arranged to have at
most one cross-engine dependency (DMA-loaded operands are "touched" first,
buffers are never reused where reuse would add a WAR wait on a DVE op).
"""

import os
import sys

for _p in (
    "/root/.axon_site",
    "/root/.axon_site/_ro/trn_rl_repo",
    "/root/.axon_site/_ro/pypackages",
    "/opt/trn_rl_repo",
):
    if os.path.isdir(_p) and _p not in sys.path:
        sys.path.append(_p)

from contextlib import ExitStack

import ml_dtypes
import numpy as np

import concourse.bass as bass
import concourse.tile as tile
from concourse import mybir
from concourse.bass_utils import run_bass_kernel_spmd

BF16 = ml_dtypes.bfloat16
E = 768
NT = 2048  # tokens
H = 8
D = 96
HC = 4  # heads per core
KC = 6  # 128-chunks over E
SC = 1.0 / float(np.sqrt(D))

_NC_CACHE = {}


def _build_bass():
    f32 = mybir.dt.float32
    bf = mybir.dt.bfloat16
    f8 = mybir.dt.float8e4
    DR = mybir.MatmulPerfMode.DoubleRow
    nc = bass.Bass(trn_type="TRN2", target_bir_lowering=False, debug=False)

    xT_d = nc.dram_tensor("xT", [128, KC, NT], bf, kind="ExternalInput").ap()
    wq_d = nc.dram_tensor("wq", [128, KC, 384], bf, kind="ExternalInput").ap()
    wk_d = nc.dram_tensor("wk", [128, KC, 384], bf, kind="ExternalInput").ap()
    wv_d = nc.dram_tensor("wv", [128, KC, 384], bf, kind="ExternalInput").ap()
    wo_d = nc.dram_tensor("wo", [128, HC, E], bf, kind="ExternalInput").ap()
    bq_d = nc.dram_tensor("bq", [128, HC], f32, kind="ExternalInput").ap()
    bk_d = nc.dram_tensor("bk", [128, HC], f32, kind="ExternalInput").ap()
    # bvz[p, h, 0:96] = bv (broadcast over p); [:, :, 96:128] = 0
    bvz_d = nc.dram_tensor("bvz", [128, HC, 128], f32, kind="ExternalInput").ap()
    yT_d = nc.dram_tensor("yT", [128, KC, NT], bf, kind="ExternalOutput").ap()

    Exp = mybir.ActivationFunctionType.Exp

    with tile.TileContext(nc) as tc, ExitStack() as ctx:
        consts = ctx.enter_context(tc.tile_pool(name="consts", bufs=1))
        big = ctx.enter_context(tc.tile_pool(name="big", bufs=1))

        wq_sb = consts.tile([128, KC, 384], bf)
        wk_sb = consts.tile([128, KC, 384], bf)
        wv_sb = consts.tile([128, KC, 384], bf)
        wo_sb = consts.tile([128, HC, E], bf)
        bq_sb = consts.tile([128, HC], f32)
        bk_sb = consts.tile([128, HC], f32)
        bvz_sb = consts.tile([128, HC, 128], f32)
        # wv + bvz first: the V projection is the first PE work and only
        # needs these (x halves are loaded right after, in the inner pool)
        nc.sync.dma_start(wv_sb, wv_d)
        nc.sync.dma_start(bvz_sb, bvz_d)
        nc.sync.dma_start(bq_sb, bq_d)
        nc.sync.dma_start(bk_sb, bk_d)

        # DVE "touches" of DMA-loaded DVE operands (consume the DMA wait here,
        # each into a distinct scratch column to avoid WAW self-waits).
        scratch = consts.tile([1, 8], f32)
        nc.vector.tensor_copy(scratch[0:1, 0:1], bq_sb[0:1, 0:1])
        nc.vector.tensor_copy(scratch[0:1, 1:2], bk_sb[0:1, 0:1])
        nc.vector.tensor_copy(scratch[0:1, 2:3], bvz_sb[0:1, 0, 0:1])
        # ACT touch of bvz (used as the exp bias operand)
        scratch_a = consts.tile([1, 1], f32)
        nc.scalar.copy(scratch_a, bvz_sb[0:1, 0, 127:128])

        # q/k in fp8e4 for DoubleRow scores: drain target [96, head, token]
        # (partition-aligned with the projection PSUM), then DMA-reshuffled
        # into [48, 2, head, token] (d split into two 48-wide k-tiles).
        qF8_sb = big.tile([96, HC, NT], f8)
        kF8_sb = big.tile([96, HC, NT], f8)
        q8_sb = big.tile([48, 2, HC, NT], f8)
        k8_sb = big.tile([48, 2, HC, NT], f8)
        # v, ones-augmented and zero-padded: [token%128, jc, head, 128]
        # cols 0:96 = v, col 96 = 1.0 (softmax row-sum), 97:128 = 0
        v_sb = big.tile([128, 16, HC, 128], bf)
        out_sb = big.tile([128, HC, NT], bf)  # [headdim(pad 128), head, token]
        scr_b = big.tile([1, 16], f32)  # per-block rb touch targets
        ypool = ctx.enter_context(tc.tile_pool(name="ypool", bufs=12))

        nc.vector.memset(v_sb[:, :, :, 96:97], 1.0)
        nc.vector.memset(v_sb[:, :, :, 97:128], 0.0)

        with (
            tc.tile_pool(name="pss", bufs=2, space="PSUM") as pss,
            tc.tile_pool(name="psu", bufs=2, space="PSUM") as psu,
            tc.tile_pool(name="epool", bufs=3) as epool,
            tc.tile_pool(name="npool", bufs=2) as npool,
            tc.tile_pool(name="drp", bufs=1, space="DRAM") as drp,
        ):
            rdram = drp.tile([HC * 4, 512], f32)

            def attn_block(h, iq):
                # one attention block = head h, query quarter iq (512 wide).
                # UT is one PSUM bank so psu can double-buffer; each scores
                # tile packs TWO key-chunks so exp stays at 1024 elem/op.
                bi = 4 * h + iq
                isl = slice(512 * iq, 512 * iq + 512)
                UT = psu.tile([128, 512], f32, tag="u")
                for jp in range(8):
                    S = pss.tile([128, 1024], f32, tag="s")
                    for n in range(2):
                        jc = 2 * jp + n
                        nc.tensor.matmul(
                            S[:, 512 * n : 512 * n + 512],
                            k8_sb[:, :, h, 128 * jc : 128 * jc + 128],
                            q8_sb[:, :, h, isl],
                            start=True,
                            stop=True,
                            perf_mode=DR,
                        )
                    Et = epool.tile([128, 1024], bf, tag="e")
                    nc.scalar.activation(Et, S, Exp, bias=bvz_sb[:, 0, 127:128])
                    for n in range(2):
                        jc = 2 * jp + n
                        nc.tensor.matmul(
                            UT,
                            v_sb[:, jc, h, :],
                            Et[:, 512 * n : 512 * n + 512],
                            start=(jp == 0 and n == 0),
                            stop=(jp == 7 and n == 1),
                        )
                # Copy UT to SBUF right away: releases the PSUM slot so the
                # next block's PV matmuls don't wait on the (long)
                # normalization chain below.
                uc = npool.tile([128, 512], f32, tag="uc")
                nc.vector.tensor_copy(uc, UT)
                # normalization: out = uc * (1/r), r = row 96 (the ones column
                # of augmented V). The partition-broadcast of 1/r goes through
                # a DRAM bounce (DMA cannot broadcast from SBUF).
                rr = npool.tile([1, 512], f32, tag="rr")
                nc.vector.reciprocal(rr, uc[96:97, :])
                nc.sync.dma_start(rdram[bi : bi + 1, :], rr)
                rb = npool.tile([128, 512], f32, tag="rb")
                row = rdram[bi : bi + 1, :]
                rr_bcast = bass.AP(
                    tensor=row.tensor,
                    offset=row.offset,
                    ap=[[0, 128]] + [list(row.ap[-1])],
                )
                nc.sync.dma_start(rb, rr_bcast)
                nc.vector.tensor_copy(scr_b[0:1, bi : bi + 1], rb[0:1, 0:1])
                nc.vector.tensor_mul(out_sb[:, h, isl], uc, rb)

            def phase3_quarter(q, psy):
                isl = slice(512 * q, 512 * q + 512)
                for mc in range(KC):
                    py = psy.tile([128, 512], f32, tag="y")
                    for c in range(HC):
                        nc.tensor.matmul(
                            py,
                            wo_sb[:, c, 128 * mc : 128 * mc + 128],
                            out_sb[:, c, isl],
                            start=(c == 0),
                            stop=(c == HC - 1),
                        )
                    y_sb = ypool.tile([128, 512], bf, tag="ysb")
                    nc.vector.tensor_copy(y_sb, py)
                    nc.sync.dma_start(yT_d[:, mc, isl], y_sb)

            with (
                tc.tile_pool(name="xpool", bufs=1) as xp,
                tc.tile_pool(name="ps1", bufs=2, space="PSUM") as ps1,
            ):
                xT_sb = xp.tile([128, KC, NT], bf)
                for quarter in range(4):
                    tsl = slice(512 * quarter, 512 * quarter + 512)
                    for kc in range(KC):
                        nc.sync.dma_start(xT_sb[:, kc, tsl], xT_d[:, kc, tsl])
                    if quarter == 1:
                        # q/k/o weights only needed after the V projection
                        nc.sync.dma_start(wq_sb, wq_d)
                        nc.sync.dma_start(wk_sb, wk_d)
                        nc.sync.dma_start(wo_sb, wo_d)

                # V projection first: every attention block needs all of V.
                for jc in range(16):
                    pv = ps1.tile([128, HC, D], f32, tag="p1")
                    for kc in range(KC):
                        nc.tensor.matmul(
                            pv,
                            xT_sb[:, kc, 128 * jc : 128 * jc + 128],
                            wv_sb[:, kc],
                            start=(kc == 0),
                            stop=(kc == KC - 1),
                        )
                    nc.vector.tensor_add(
                        v_sb[:, jc, :, 0:96], pv, bvz_sb[:, :, 0:96]
                    )

                # Per-head QK projection immediately followed by that head's
                # attention, so ScalarE (exp) starts early and the remaining
                # heads' projections overlap the ACT-bound attention.
                for h in range(HC):
                    for i in range(4):
                        isl = slice(512 * i, 512 * i + 512)
                        pq = ps1.tile([128, 512], f32, tag="p1")
                        for kc in range(KC):
                            nc.tensor.matmul(
                                pq[0:96, :],
                                wq_sb[:, kc, 96 * h : 96 * h + 96],
                                xT_sb[:, kc, isl],
                                start=(kc == 0),
                                stop=(kc == KC - 1),
                            )
                        nc.vector.tensor_scalar_add(
                            qF8_sb[:, h, isl], pq[0:96, :], bq_sb[0:96, h : h + 1]
                        )
                        nc.sync.dma_start(q8_sb[:, 0, h, isl], qF8_sb[0:48, h, isl])
                        nc.sync.dma_start(q8_sb[:, 1, h, isl], qF8_sb[48:96, h, isl])
                        pk = ps1.tile([128, 512], f32, tag="p1")
                        for kc in range(KC):
                            nc.tensor.matmul(
                                pk[0:96, :],
                                wk_sb[:, kc, 96 * h : 96 * h + 96],
                                xT_sb[:, kc, isl],
                                start=(kc == 0),
                                stop=(kc == KC - 1),
                            )
                        nc.vector.tensor_scalar_add(
                            kF8_sb[:, h, isl], pk[0:96, :], bk_sb[0:96, h : h + 1]
                        )
                        nc.sync.dma_start(k8_sb[:, 0, h, isl], kF8_sb[0:48, h, isl])
                        nc.sync.dma_start(k8_sb[:, 1, h, isl], kF8_sb[48:96, h, isl])
                    if h < HC - 1:
                        attn_block(h, 0)
                        attn_block(h, 1)

            # ps1/xpool closed: 2 PSUM banks free for early output projection.
            with tc.tile_pool(name="psy0", bufs=2, space="PSUM") as psy0:
                attn_block(HC - 1, 0)
                attn_block(HC - 1, 1)
                for h in range(HC):
                    attn_block(h, 2)
                # quarters 0/1 are complete; project them under the iq=2/3
                # attention blocks (emitted later = lower priority = fillers).
                phase3_quarter(0, psy0)
                for h in range(HC):
                    attn_block(h, 3)
                phase3_quarter(1, psy0)
                phase3_quarter(2, psy0)

        # remaining PSUM free: last quarter fully pipelined
        with tc.tile_pool(name="psy1", bufs=4, space="PSUM") as psy1:
            phase3_quarter(3, psy1)

    _split_multi_waits(nc)
    return nc


def _split_multi_waits(nc):
    """Walrus codegen allows only ONE sync wait on most compute-instruction
    structs. Hoist extra waits onto standalone EventSemaphore instructions
    inserted just before the offender on the same engine (semantically
    identical for in-order engines). DMA descriptors (queue-dispatched) are
    left alone."""
    import bass_rust

    n_split = 0
    for f in nc.m.functions:
        for blk in f.blocks:
            il = blk.instructions
            i = 0
            while i < len(il):
                inst = il[i]
                try:
                    si = inst.sync_info
                    waits = list(si.on_wait)
                except Exception:
                    i += 1
                    continue
                if len(waits) > 1 and inst.engine != mybir.EngineType.Unassigned:
                    for w in waits[:-1]:
                        ev = mybir.InstEventSemaphore(
                            name=f"wsplit_{n_split}", ins=[], outs=[]
                        )
                        n_split += 1
                        ev.engine = inst.engine
                        ev.sync_info = bass_rust.SyncInfo(on_wait=[w], on_update=[])
                        il.insert(i, ev)
                        i += 1
                    inst.sync_info = bass_rust.SyncInfo(
                        on_wait=[waits[-1]], on_update=list(si.on_update)
                    )
                i += 1
    return n_split


def _get_nc():
    if "nc" not in _NC_CACHE:
        _NC_CACHE["nc"] = _build_bass()
    return _NC_CACHE["nc"]


def _to_lhsT(w):
    """[384, 768] weight (rows = output dims) -> [128, KC, 384] bf16 lhsT chunks."""
    return np.ascontiguousarray(
        w.T.reshape(KC, 128, 384).transpose(1, 0, 2)
    ).astype(BF16)


def _prep_half(Wq, bq, Wk, bk, Wv, bv, Wo, half):
    sl = slice(384 * half, 384 * (half + 1))
    wq_l = _to_lhsT(Wq[sl, :].astype(np.float32) * SC)
    wk_l = _to_lhsT(Wk[sl, :].astype(np.float32))
    wv_l = _to_lhsT(Wv[sl, :].astype(np.float32))

    WoT = Wo[:, sl].T.astype(np.float32)  # [384, 768]
    wo_pad = np.zeros((HC, 128, E), np.float32)
    for h in range(HC):
        wo_pad[h, 0:96] = WoT[96 * h : 96 * h + 96]
    wo_l = np.ascontiguousarray(wo_pad.transpose(1, 0, 2)).astype(BF16)

    def bias4(b, scale=1.0):
        out = np.zeros((128, HC), np.float32)
        bb = b[sl].astype(np.float32) * scale
        for h in range(HC):
            out[0:96, h] = bb[96 * h : 96 * h + 96]
        return out

    bvz = np.zeros((128, HC, 128), np.float32)
    bvz[:, :, 0:96] = np.asarray(bv)[sl].astype(np.float32).reshape(HC, D)[None]

    return dict(
        wq=wq_l, wk=wk_l, wv=wv_l, wo=wo_l,
        bq=bias4(bq, SC), bk=bias4(bk), bvz=bvz,
    )


def _run(x, Wq, bq, Wk, bk, Wv, bv, Wo, bo, trace=False):
    x = np.asarray(x, dtype=np.float32)
    B = x.shape[0]
    halves = [
        _prep_half(np.asarray(Wq), np.asarray(bq), np.asarray(Wk), np.asarray(bk),
                   np.asarray(Wv), np.asarray(bv), np.asarray(Wo), hf)
        for hf in range(2)
    ]
    xTs = []
    for b in range(B):
        xT = np.ascontiguousarray(
            x[b].T.reshape(KC, 128, NT).transpose(1, 0, 2)
        ).astype(BF16)
        xTs.append(xT)

    in_maps = []
    for c in range(8):
        b, hf = c // 2, c % 2
        m = dict(halves[hf])
        m["xT"] = xTs[b]
        in_maps.append(m)

    nc = _get_nc()
    res = run_bass_kernel_spmd(nc, in_maps, core_ids=list(range(8)), trace=trace)

    bo32 = np.asarray(bo, dtype=np.float32)
    y = np.empty((B, NT, E), np.float32)
    for b in range(B):
        p0 = res.results[2 * b]["yT"].astype(np.float32).transpose(1, 0, 2).reshape(E, NT)
        p1 = res.results[2 * b + 1]["yT"].astype(np.float32).transpose(1, 0, 2).reshape(E, NT)
        y[b] = (p0 + p1).T + bo32
    return y, res


def kernel(x, Wq, bq, Wk, bk, Wv, bv, Wo, bo):
    y, _ = _run(x, Wq, bq, Wk, bk, Wv, bv, Wo, bo, trace=False)
    return y

